# revision 4
# baseline (speedup 1.0000x reference)
"""SNN (soft-nearest-neighbor) contrastive loss on 8 Trainium2 NeuronCores.

Math
----
z = concat(x, y) in R^{8192x128};  d_ij = ||z_i - z_j||.
Reference computes, per row i:  softmax-style ratio
    loss_i = -log( exp(s_pair - m_i) / sum_{j != i} exp(s_ij - m_i) + tiny ),
with s_ij = -d_ij.  The row-max m_i cancels mathematically, so we compute
    S0_i  = sum_{j != i} exp(-d_ij)          (device, fused)
    dp_i  = d_{i, pair(i)}                   (device)
    loss  = mean_i( -log( exp(-dp_i)/S0_i + tiny ) )   (host, trivial)

Device strategy (one SPMD program, 8 cores, rows sharded 1024/core)
------------------------------------------------------------------
* PE assembles d2 = ||z_i||^2 + ||z_j||^2 - 2 z_i.z_j entirely in PSUM:
  - main matmul  u^T u  with u = bf16(sqrt(2) z)  (bf16 @ full rate)
  - K=2 accumulate matmul adds -||u_j||^2/2 (hi/lo bf16 split for f32 accuracy)
  - a tiny identity matmul adds -16384 on the diagonal so exp -> 0 ("nuke")
  - the row term ||u_i||^2/2 is folded into the ACT bias.
* ACT pass 1: w = Sqrt(-PSUM + bias)  -> d_ij tile
  ACT pass 2: Exp(-w) with fused accum_out -> row sums S0 (no reduce pass)
  Sqrt and Exp live in different ACT table sets (~2.7us swap), so row
  subtiles are processed in batches with all Sqrts before all Exps.
* Each core receives column-ROTATED operand copies (rotated by its row
  offset) so the diagonal lands at a compile-time-constant tile position:
  one identical program for all 8 cores, no partition-id, no collectives.
"""

import os
import sys
from contextlib import ExitStack

import numpy as np

_TRN_REPO = os.environ.get("TRN_RL_REPO", "/opt/trn_rl_repo")
if _TRN_REPO not in sys.path:
    sys.path.insert(0, _TRN_REPO)

import ml_dtypes

BF16 = ml_dtypes.bfloat16

B = 4096
D = 128
N = 2 * B            # 8192 rows of z
NCORES = 8
RPC = N // NCORES    # 1024 rows per core
S = RPC // 128       # 8 row-subtiles per core
CT = 512             # matmul moving tile (one PSUM bank)
PS = 2048            # PSUM tile columns (4 banks)
NPT = N // PS        # 4 PSUM tiles per row strip
LARGE = 16384.0      # diagonal nuke: d2 -> 16384, d -> 128, exp(-128) -> 0
BATCH = 3            # row-subtiles per ACT table phase

# Set True (e.g. from test harness) to run with NTFF tracing and stash the
# BassKernelResults on LAST_RESULT.
PROFILE = False
LAST_RESULT = None

_cache = {}


def _build_program():
    import concourse.tile as tile
    from concourse import bacc, mybir

    f32 = mybir.dt.float32
    bf16 = mybir.dt.bfloat16
    AF = mybir.ActivationFunctionType
    OP = mybir.AluOpType

    nc = bacc.Bacc()

    h_ubtr = nc.declare_dram_parameter("ubtr", [128, N], bf16, isOutput=False)
    h_hnegr = nc.declare_dram_parameter("hnegr", [2, N], bf16, isOutput=False)
    h_dfix = nc.declare_dram_parameter("dfix", [128, 4 * CT], bf16, isOutput=False)
    h_ident = nc.declare_dram_parameter("ident", [128, 128], bf16, isOutput=False)
    h_hsqp = nc.declare_dram_parameter("hsqp", [128, S], f32, isOutput=False)
    h_zrow = nc.declare_dram_parameter("zrow", [128, S * 128], f32, isOutput=False)
    h_zpair = nc.declare_dram_parameter("zpair", [128, S * 128], f32, isOutput=False)
    h_sqr = nc.declare_dram_parameter("sqr", [128, S], f32, isOutput=False)
    h_sqpair = nc.declare_dram_parameter("sqpair", [128, S], f32, isOutput=False)
    h_s0 = nc.declare_dram_parameter("s0", [128, S], f32, isOutput=True)
    h_dp = nc.declare_dram_parameter("dp", [128, S], f32, isOutput=True)

    with tile.TileContext(nc) as tc, ExitStack() as ctx:
        const = ctx.enter_context(tc.tile_pool(name="const", bufs=1))
        wpool = ctx.enter_context(tc.tile_pool(name="wbuf", bufs=BATCH))
        pspool = ctx.enter_context(tc.tile_pool(name="ps", bufs=2, space="PSUM"))
        misc = ctx.enter_context(tc.tile_pool(name="misc", bufs=2))
        dumpp = ctx.enter_context(tc.tile_pool(name="dump", bufs=1))

        t_ubtr = const.tile([128, N], bf16)
        nc.sync.dma_start(out=t_ubtr[:], in_=h_ubtr[:])
        t_hnegr = const.tile([2, N], bf16)
        nc.sync.dma_start(out=t_hnegr[:], in_=h_hnegr[:])
        t_dfix = const.tile([128, 4 * CT], bf16)
        nc.sync.dma_start(out=t_dfix[:], in_=h_dfix[:])
        t_ident = const.tile([128, 128], bf16)
        nc.sync.dma_start(out=t_ident[:], in_=h_ident[:])
        t_hsqp = const.tile([128, S], f32)
        nc.sync.dma_start(out=t_hsqp[:], in_=h_hsqp[:])
        t_zrow = const.tile([128, S * 128], f32)
        nc.sync.dma_start(out=t_zrow[:], in_=h_zrow[:])
        t_zpair = const.tile([128, S * 128], f32)
        nc.sync.dma_start(out=t_zpair[:], in_=h_zpair[:])
        t_sqr = const.tile([128, S], f32)
        nc.sync.dma_start(out=t_sqr[:], in_=h_sqr[:])
        t_sqpair = const.tile([128, S], f32)
        nc.sync.dma_start(out=t_sqpair[:], in_=h_sqpair[:])

        t_ones2 = const.tile([2, 128], bf16)
        nc.vector.memset(t_ones2[:], 1.0)

        s0_t = const.tile([128, S], f32)
        dp_t = const.tile([128, S], f32)

        for b0 in range(0, S, BATCH):
            batch = list(range(b0, min(b0 + BATCH, S)))
            ws = {}
            # ---- Sqrt phase (and PE matmuls) for the whole batch ----
            for s in batch:
                w = wpool.tile([128, N], f32, tag="w")
                ws[s] = w
                for t in range(NPT):
                    ps = pspool.tile([128, PS], f32, tag="ps")
                    for q in range(4):
                        ct = 4 * t + q
                        nc.tensor.matmul(
                            ps[:, q * CT:(q + 1) * CT],
                            t_ubtr[:, s * 128:(s + 1) * 128],
                            t_ubtr[:, ct * CT:(ct + 1) * CT],
                            start=True,
                            stop=False,
                        )
                    for q in range(4):
                        ct = 4 * t + q
                        # rotated columns put this subtile's diagonal at
                        # column tile s//4, local offset 128*(s%4)
                        is_diag = ct == s // 4
                        nc.tensor.matmul(
                            ps[:, q * CT:(q + 1) * CT],
                            t_ones2[:],
                            t_hnegr[:, ct * CT:(ct + 1) * CT],
                            start=False,
                            stop=not is_diag,
                        )
                        if is_diag:
                            k = s % 4
                            nc.tensor.matmul(
                                ps[:, q * CT:(q + 1) * CT],
                                t_ident[:],
                                t_dfix[:, k * CT:(k + 1) * CT],
                                start=False,
                                stop=True,
                            )
                    # w = sqrt(-(PSUM) + ||u_i||^2/2) = d_ij
                    nc.scalar.activation(
                        out=w[:, t * PS:(t + 1) * PS],
                        in_=ps[:],
                        func=AF.Sqrt,
                        bias=t_hsqp[:, s:s + 1],
                        scale=-1.0,
                    )
                # ---- numerator (pair distance), f32, tiny ----
                junk = misc.tile([128, 128], f32, tag="junk")
                dot = misc.tile([128, 1], f32, tag="dot")
                tq = misc.tile([128, 1], f32, tag="tq")
                nc.vector.tensor_mul(
                    junk[:],
                    t_zrow[:, s * 128:(s + 1) * 128],
                    t_zpair[:, s * 128:(s + 1) * 128],
                )
                nc.vector.tensor_reduce(
                    out=dot[:],
                    in_=junk[:],
                    axis=mybir.AxisListType.X,
                    op=OP.add,
                )
                nc.vector.tensor_scalar(
                    out=tq[:],
                    in0=dot[:],
                    scalar1=-2.0,
                    scalar2=t_sqr[:, s:s + 1],
                    op0=OP.mult,
                    op1=OP.add,
                )
                nc.scalar.activation(
                    out=dp_t[:, s:s + 1],
                    in_=tq[:],
                    func=AF.Sqrt,
                    bias=t_sqpair[:, s:s + 1],
                    scale=1.0,
                )
            # ---- Exp phase for the whole batch ----
            for s in batch:
                dump = dumpp.tile([128, N], bf16, tag="dump")
                nc.scalar.activation(
                    out=dump[:],
                    in_=ws[s][:],
                    func=AF.Exp,
                    scale=-1.0,
                    accum_out=s0_t[:, s:s + 1],
                )

        nc.sync.dma_start(out=h_s0[:], in_=s0_t[:])
        nc.sync.dma_start(out=h_dp[:], in_=dp_t[:])

    nc.finalize()
    return nc


def get_program():
    if "nc" not in _cache:
        _cache["nc"] = _build_program()
    return _cache["nc"]


def make_in_maps(x, y):
    """Host-side prep: build the per-core (column-rotated) operand arrays."""
    x = np.asarray(x, dtype=np.float32)
    y = np.asarray(y, dtype=np.float32)
    z = np.concatenate([x, y], axis=0)  # [N, D]

    u = (np.float32(np.sqrt(2.0)) * z).astype(BF16)
    uf = u.astype(np.float32)
    hsq = np.float32(0.5) * (uf * uf).sum(axis=1, dtype=np.float32)  # ||u||^2/2
    h1 = hsq.astype(BF16)
    h2 = (hsq - h1.astype(np.float32)).astype(BF16)
    hneg = np.stack([-h1, -h2])  # [2, N] bf16
    sq = (z * z).sum(axis=1, dtype=np.float32)

    ubt = np.ascontiguousarray(u.T)  # [D, N] bf16

    dfix = np.zeros((128, 4 * CT), dtype=BF16)
    idx = np.arange(128)
    for k in range(4):
        dfix[idx, 512 * k + 128 * k + idx] = BF16(-LARGE)
    ident = np.eye(128, dtype=BF16)

    pair = np.concatenate([np.arange(B) + B, np.arange(B)])

    in_maps = []
    for c in range(NCORES):
        r0 = c * RPC
        rows = np.arange(r0, r0 + RPC)
        prow = pair[rows]

        def rot(a):
            return np.ascontiguousarray(np.roll(a, -r0, axis=-1))

        def pcol(vec, sel):  # [RPC] values -> [128, S] per-partition layout
            return np.ascontiguousarray(vec[sel].reshape(S, 128).T)

        def prowmat(mat, sel):  # [RPC, D] -> [128, S*128]
            return np.ascontiguousarray(
                mat[sel].reshape(S, 128, D).transpose(1, 0, 2).reshape(128, S * D)
            )

        in_maps.append(
            {
                "ubtr": rot(ubt),
                "hnegr": rot(hneg),
                "dfix": dfix,
                "ident": ident,
                "hsqp": pcol(hsq, rows),
                "zrow": prowmat(z, rows),
                "zpair": prowmat(z, prow),
                "sqr": pcol(sq, rows),
                "sqpair": pcol(sq, prow),
            }
        )
    return in_maps


def finish_on_host(results):
    """Gather per-core S0/dp and compute the final scalar loss."""
    S0 = np.empty(N, dtype=np.float64)
    DP = np.empty(N, dtype=np.float64)
    for c in range(NCORES):
        s0 = np.asarray(results[c]["s0"], dtype=np.float64)  # [128, S]
        dp = np.asarray(results[c]["dp"], dtype=np.float64)
        S0[c * RPC:(c + 1) * RPC] = s0.T.reshape(-1)
        DP[c * RPC:(c + 1) * RPC] = dp.T.reshape(-1)
    tiny = float(np.finfo(np.float32).tiny)
    num = np.exp(-DP)
    loss = -np.log(num / S0 + tiny)
    return np.asarray(loss.mean(), dtype=np.float32)


def kernel(x, y):
    global LAST_RESULT
    from concourse.bass_utils import run_bass_kernel_spmd

    nc = get_program()
    in_maps = make_in_maps(x, y)
    res = run_bass_kernel_spmd(
        nc, in_maps, list(range(NCORES)), trace=PROFILE
    )
    LAST_RESULT = res
    return finish_on_host(res.results)


# revision 5
# speedup vs baseline: 1.1341x; 1.1341x over previous
"""SNN (soft-nearest-neighbor) contrastive loss on 8 Trainium2 NeuronCores.

Math
----
z = concat(x, y) in R^{8192x128};  d_ij = ||z_i - z_j||.
Reference computes, per row i, a softmax-style ratio with the row max
subtracted; the max cancels mathematically, so we compute
    S0_i  = sum_{j != i} exp(-d_ij)          (device, fused row-sums)
    dp_i  = d_{i, pair(i)}                   (device)
    loss  = mean_i( -log( exp(-dp_i)/S0_i + tiny ) )   (host, trivial)

Device strategy (one SPMD program, 8 cores, rows sharded 1024/core)
------------------------------------------------------------------
* PE: bf16 matmul u^T u (u = bf16(sqrt(2) z)) into PSUM, plus a tiny
  identity matmul that adds -16384 on the diagonal so exp -> 0 ("nuke").
* DVE: v = (PSUM - ||u_i||^2/2) - ||u_j||^2/2  (scalar_tensor_tensor with a
  per-partition scalar and a broadcast row tile) = -d2.
* ACT pass 1: w = Sqrt(-v) = d_ij; pass 2: Exp(-w) in-place with fused
  accum_out giving the row sums S0. Sqrt and Exp live in different ACT
  table sets (~1.3us swap), so row subtiles are processed in batches with
  all Sqrts before all Exps.
* Each core receives column-ROTATED operand copies (rotated by its row
  offset) so the diagonal lands at a compile-time-constant tile position:
  one identical program for all 8 cores, no partition-id, no collectives.
"""

import os
import sys
from contextlib import ExitStack

import numpy as np

_TRN_REPO = os.environ.get("TRN_RL_REPO", "/opt/trn_rl_repo")
if _TRN_REPO not in sys.path:
    sys.path.insert(0, _TRN_REPO)

import ml_dtypes

BF16 = ml_dtypes.bfloat16

B = 4096
D = 128
N = 2 * B            # 8192 rows of z
NCORES = 8
RPC = N // NCORES    # 1024 rows per core
S = RPC // 128       # 8 row-subtiles per core
CT = 512             # matmul moving tile (one PSUM bank)
PS = 2048            # PSUM tile columns (4 banks)
NPT = N // PS        # 4 PSUM tiles per row strip
LARGE = 16384.0      # diagonal nuke: d2 -> 16384, d -> 128, exp(-128) -> 0
BATCH = 3            # row-subtiles per ACT table phase

PROFILE = False
LAST_RESULT = None

_cache = {}


def _build_program():
    import concourse.tile as tile
    from concourse import bacc, mybir

    f32 = mybir.dt.float32
    bf16 = mybir.dt.bfloat16
    AF = mybir.ActivationFunctionType
    OP = mybir.AluOpType

    nc = bacc.Bacc()

    h_ubtr = nc.declare_dram_parameter("ubtr", [128, N], bf16, isOutput=False)
    h_hsqjb = nc.declare_dram_parameter("hsqjb", [128, N], f32, isOutput=False)
    h_dfix = nc.declare_dram_parameter("dfix", [128, 4 * CT], bf16, isOutput=False)
    h_ident = nc.declare_dram_parameter("ident", [128, 128], bf16, isOutput=False)
    h_hsqp = nc.declare_dram_parameter("hsqp", [128, S], f32, isOutput=False)
    h_zrow = nc.declare_dram_parameter("zrow", [128, S * 128], f32, isOutput=False)
    h_zpair = nc.declare_dram_parameter("zpair", [128, S * 128], f32, isOutput=False)
    h_sqr = nc.declare_dram_parameter("sqr", [128, S], f32, isOutput=False)
    h_sqpair = nc.declare_dram_parameter("sqpair", [128, S], f32, isOutput=False)
    h_s0 = nc.declare_dram_parameter("s0", [128, S], f32, isOutput=True)
    h_dp = nc.declare_dram_parameter("dp", [128, S], f32, isOutput=True)

    with tile.TileContext(nc) as tc, ExitStack() as ctx:
        const = ctx.enter_context(tc.tile_pool(name="const", bufs=1))
        wpool = ctx.enter_context(tc.tile_pool(name="wbuf", bufs=BATCH))
        vpool = ctx.enter_context(tc.tile_pool(name="vbuf", bufs=3))
        pspool = ctx.enter_context(tc.tile_pool(name="ps", bufs=2, space="PSUM"))
        misc = ctx.enter_context(tc.tile_pool(name="misc", bufs=2))

        # small constants first so the first matmuls / DVE ops aren't gated
        # on the big transfers
        t_dfix = const.tile([128, 4 * CT], bf16)
        nc.sync.dma_start(out=t_dfix[:], in_=h_dfix[:])
        t_ident = const.tile([128, 128], bf16)
        nc.sync.dma_start(out=t_ident[:], in_=h_ident[:])
        t_hsqp = const.tile([128, S], f32)
        nc.sync.dma_start(out=t_hsqp[:], in_=h_hsqp[:])
        t_sqr = const.tile([128, S], f32)
        nc.sync.dma_start(out=t_sqr[:], in_=h_sqr[:])
        t_sqpair = const.tile([128, S], f32)
        nc.sync.dma_start(out=t_sqpair[:], in_=h_sqpair[:])

        # big operands, chunked so early tiles unblock quickly
        t_ubtr = const.tile([128, N], bf16)
        for ch in range(NPT):
            nc.sync.dma_start(
                out=t_ubtr[:, ch * PS:(ch + 1) * PS],
                in_=h_ubtr[:, ch * PS:(ch + 1) * PS],
            )
        t_hsqjb = const.tile([128, N], f32)
        for ch in range(NPT):
            nc.sync.dma_start(
                out=t_hsqjb[:, ch * PS:(ch + 1) * PS],
                in_=h_hsqjb[:, ch * PS:(ch + 1) * PS],
            )
        t_zrow = const.tile([128, S * 128], f32)
        nc.sync.dma_start(out=t_zrow[:], in_=h_zrow[:])
        t_zpair = const.tile([128, S * 128], f32)
        nc.sync.dma_start(out=t_zpair[:], in_=h_zpair[:])

        s0_t = const.tile([128, S], f32)
        dp_t = const.tile([128, S], f32)

        for b0 in range(0, S, BATCH):
            batch = list(range(b0, min(b0 + BATCH, S)))
            ws = {}
            # ---- Sqrt phase (PE matmuls -> DVE d2 assembly -> ACT sqrt) ----
            for s in batch:
                w = wpool.tile([128, N], f32, tag="w")
                ws[s] = w
                for t in range(NPT):
                    ps = pspool.tile([128, PS], f32, tag="ps")
                    for q in range(4):
                        ct = 4 * t + q
                        # rotated columns put this subtile's diagonal at
                        # column tile s//4, local offset 128*(s%4)
                        is_diag = ct == s // 4
                        nc.tensor.matmul(
                            ps[:, q * CT:(q + 1) * CT],
                            t_ubtr[:, s * 128:(s + 1) * 128],
                            t_ubtr[:, ct * CT:(ct + 1) * CT],
                            start=True,
                            stop=not is_diag,
                        )
                        if is_diag:
                            k = s % 4
                            nc.tensor.matmul(
                                ps[:, q * CT:(q + 1) * CT],
                                t_ident[:],
                                t_dfix[:, k * CT:(k + 1) * CT],
                                start=False,
                                stop=True,
                            )
                    # v = (P - ||u_i||^2/2) - ||u_j||^2/2 = -d2
                    v = vpool.tile([128, PS], f32, tag="v")
                    nc.vector.scalar_tensor_tensor(
                        out=v[:],
                        in0=ps[:],
                        scalar=t_hsqp[:, s:s + 1],
                        in1=t_hsqjb[:, t * PS:(t + 1) * PS],
                        op0=OP.subtract,
                        op1=OP.subtract,
                    )
                    # w = sqrt(-v) = d_ij
                    nc.scalar.activation(
                        out=w[:, t * PS:(t + 1) * PS],
                        in_=v[:],
                        func=AF.Sqrt,
                        scale=-1.0,
                    )
                # ---- numerator (pair distance), f32, tiny ----
                junk = misc.tile([128, 128], f32, tag="junk")
                dot = misc.tile([128, 1], f32, tag="dot")
                tq = misc.tile([128, 1], f32, tag="tq")
                nc.vector.tensor_mul(
                    junk[:],
                    t_zrow[:, s * 128:(s + 1) * 128],
                    t_zpair[:, s * 128:(s + 1) * 128],
                )
                nc.vector.tensor_reduce(
                    out=dot[:], in_=junk[:], axis=mybir.AxisListType.X, op=OP.add,
                )
                nc.vector.tensor_scalar(
                    out=tq[:], in0=dot[:], scalar1=-2.0,
                    scalar2=t_sqr[:, s:s + 1], op0=OP.mult, op1=OP.add,
                )
                nc.scalar.activation(
                    out=dp_t[:, s:s + 1], in_=tq[:], func=AF.Sqrt,
                    bias=t_sqpair[:, s:s + 1], scale=1.0,
                )
            # ---- Exp phase for the whole batch (in-place over w) ----
            for s in batch:
                nc.scalar.activation(
                    out=ws[s][:],
                    in_=ws[s][:],
                    func=AF.Exp,
                    scale=-1.0,
                    accum_out=s0_t[:, s:s + 1],
                )

        nc.sync.dma_start(out=h_s0[:], in_=s0_t[:])
        nc.sync.dma_start(out=h_dp[:], in_=dp_t[:])

    nc.finalize()
    return nc


def get_program():
    if "nc" not in _cache:
        _cache["nc"] = _build_program()
    return _cache["nc"]


def make_in_maps(x, y):
    """Host-side prep: build the per-core (column-rotated) operand arrays."""
    x = np.asarray(x, dtype=np.float32)
    y = np.asarray(y, dtype=np.float32)
    z = np.concatenate([x, y], axis=0)  # [N, D]

    u = (np.float32(np.sqrt(2.0)) * z).astype(BF16)
    uf = u.astype(np.float32)
    hsq = np.float32(0.5) * (uf * uf).sum(axis=1, dtype=np.float32)  # ||u||^2/2
    sq = (z * z).sum(axis=1, dtype=np.float32)

    ubt = np.ascontiguousarray(u.T)  # [D, N] bf16

    dfix = np.zeros((128, 4 * CT), dtype=BF16)
    idx = np.arange(128)
    for k in range(4):
        dfix[idx, 512 * k + 128 * k + idx] = BF16(-LARGE)
    ident = np.eye(128, dtype=BF16)

    pair = np.concatenate([np.arange(B) + B, np.arange(B)])

    in_maps = []
    for c in range(NCORES):
        r0 = c * RPC
        rows = np.arange(r0, r0 + RPC)
        prow = pair[rows]

        def rot(a):
            return np.ascontiguousarray(np.roll(a, -r0, axis=-1))

        def pcol(vec, sel):  # [RPC] values -> [128, S] per-partition layout
            return np.ascontiguousarray(vec[sel].reshape(S, 128).T)

        def prowmat(mat, sel):  # [RPC, D] -> [128, S*128]
            return np.ascontiguousarray(
                mat[sel].reshape(S, 128, D).transpose(1, 0, 2).reshape(128, S * D)
            )

        hsq_rot = np.roll(hsq, -r0)
        in_maps.append(
            {
                "ubtr": rot(ubt),
                "hsqjb": np.ascontiguousarray(
                    np.broadcast_to(hsq_rot[None, :], (128, N))
                ),
                "dfix": dfix,
                "ident": ident,
                "hsqp": pcol(hsq, rows),
                "zrow": prowmat(z, rows),
                "zpair": prowmat(z, prow),
                "sqr": pcol(sq, rows),
                "sqpair": pcol(sq, prow),
            }
        )
    return in_maps


def finish_on_host(results):
    """Gather per-core S0/dp and compute the final scalar loss."""
    S0 = np.empty(N, dtype=np.float64)
    DP = np.empty(N, dtype=np.float64)
    for c in range(NCORES):
        s0 = np.asarray(results[c]["s0"], dtype=np.float64)  # [128, S]
        dp = np.asarray(results[c]["dp"], dtype=np.float64)
        S0[c * RPC:(c + 1) * RPC] = s0.T.reshape(-1)
        DP[c * RPC:(c + 1) * RPC] = dp.T.reshape(-1)
    tiny = float(np.finfo(np.float32).tiny)
    num = np.exp(-DP)
    loss = -np.log(num / S0 + tiny)
    return np.asarray(loss.mean(), dtype=np.float32)


def kernel(x, y):
    global LAST_RESULT
    from concourse.bass_utils import run_bass_kernel_spmd

    nc = get_program()
    in_maps = make_in_maps(x, y)
    res = run_bass_kernel_spmd(
        nc, in_maps, list(range(NCORES)), trace=PROFILE
    )
    LAST_RESULT = res
    return finish_on_host(res.results)
